# revision 3
# baseline (speedup 1.0000x reference)
"""Trainium2 Bass kernel for nn_InteractionNetwork (GNN message passing).

Strategy (8 NeuronCores, SPMD):
  - Sort edges by destination node on host; shard edges across cores by
    destination-node range (each core owns a contiguous slice of nodes and
    ALL edges pointing into it). This makes the per-core partial
    segment_sum complete for its node slice -> NO large all-reduce of the
    [N, D] aggregate is needed at all.
  - Edge MLP layer 1 is decomposed:  concat(src,dst,ea) @ We1 =
    x@W_src (gathered by src) + x@W_dst + be1 (gathered by dst) + ea@W_e.
    The x@W halves are precomputed per-node on device ("h tables"), so the
    per-edge work is two row gathers (indirect DMA) + one small matmul.
  - Per 128-edge tile, z1 is accumulated in PSUM h-major; layer 2 and the
    destination-block segment-sum are single matmuls using a one-hot
    selection matrix built with is_equal (edges are destination-sorted, so
    each tile's destinations live inside one 128-node block).
  - Per-graph mean pooling uses the same one-hot matmul trick on the
    (sorted) batch vector; the tiny [512,128] pooled partials are
    all-reduced; the global MLP is computed redundantly on every core.

All data-dependent values (indices, per-tile one-hot ids, paddings) are
shipped as per-core *input data* so one SPMD program serves all cores.
"""

import math
from contextlib import ExitStack

import numpy as np

import concourse.bacc as bacc
import concourse.bass as bass
import concourse.mybir as mybir
import concourse.tile as tile
from concourse.bass import IndirectOffsetOnAxis
from concourse.bass_utils import run_bass_kernel_spmd

F32 = mybir.dt.float32
I32 = mybir.dt.int32
AF = mybir.ActivationFunctionType
ALU = mybir.AluOpType

P = 128


def default_dims():
    return dict(
        n_cores=8,
        n_nodes=50000,
        n_edges=800000,
        n_graphs=512,
        d_in=128,
        d_edge=32,
        d_hid=128,
        d_glob=64,
    )


def _derive(dims):
    d = dict(dims)
    nc_ = d["n_cores"]
    # nodes per core, rounded up to a multiple of P
    npc = math.ceil(d["n_nodes"] / nc_ / P) * P
    d["npc"] = npc
    d["nb"] = npc // P
    d["npad"] = npc * nc_
    d["gp"] = d["n_graphs"] + 8  # pooled rows + dump rows for scatter
    assert d["d_in"] == P and d["d_hid"] == P
    assert d["n_graphs"] % P == 0 or d["n_graphs"] < P
    return d


# --------------------------------------------------------------------------
# host-side input preparation
# --------------------------------------------------------------------------

def prep_inputs(inputs, dims):
    d = _derive(dims)
    ncores, npc, nb, npad = d["n_cores"], d["npc"], d["nb"], d["npad"]
    N, E, G, GP = d["n_nodes"], d["n_edges"], d["n_graphs"], d["gp"]
    DE, DG = d["d_edge"], d["d_glob"]

    x = np.asarray(inputs["x"], np.float32)
    edge_index = np.asarray(inputs["edge_index"]).astype(np.int64)
    ea = np.asarray(inputs["edge_attr"], np.float32)
    u = np.asarray(inputs["u"], np.float32)
    batch = np.asarray(inputs["batch"]).astype(np.int64)
    We1 = np.asarray(inputs["We1"], np.float32)
    be1 = np.asarray(inputs["be1"], np.float32)
    We2 = np.asarray(inputs["We2"], np.float32)
    be2 = np.asarray(inputs["be2"], np.float32)
    Wg = np.asarray(inputs["Wg"], np.float32)
    bg = np.asarray(inputs["bg"], np.float32)
    Wn = np.asarray(inputs["Wn"], np.float32)
    bn = np.asarray(inputs["bn"], np.float32)
    Wu = np.asarray(inputs["Wu"], np.float32)
    bu = np.asarray(inputs["bu"], np.float32)

    src = edge_index[0]
    dst = edge_index[1]
    perm = np.argsort(dst, kind="stable")
    src_s = src[perm].astype(np.int64)
    dst_s = dst[perm].astype(np.int64)
    ea_s = ea[perm]

    nblocks = ncores * nb
    blk = dst_s // P
    cnt = np.bincount(blk, minlength=nblocks)
    C = max(1, int(math.ceil(cnt.max() / P)))
    cap = C * P

    starts = np.zeros(nblocks + 1, np.int64)
    np.cumsum(cnt, out=starts[1:])
    pos = np.arange(E, dtype=np.int64) - starts[blk]

    src_pad = np.zeros((nblocks, cap), np.int32)
    dl_pad = np.full((nblocks, cap), 1.0e9, np.float32)
    dg_pad = np.zeros((nblocks, cap), np.int32)
    ea_pad = np.zeros((nblocks, cap, DE), np.float32)
    src_pad[blk, pos] = src_s.astype(np.int32)
    dl_pad[blk, pos] = (dst_s - blk * P).astype(np.float32)
    dg_pad[blk, pos] = dst_s.astype(np.int32)
    ea_pad[blk, pos, :] = ea_s

    # per-tile-major layouts:
    #   idx/dl:  [cores, nb, P, C]   (column t = tile t of the block)
    #   ea:      [cores, nb, DE, C*P] (slab [:, t*P:(t+1)*P] = tile t, T)
    src_t = src_pad.reshape(ncores, nb, C, P).transpose(0, 1, 3, 2).copy()
    dg_t = dg_pad.reshape(ncores, nb, C, P).transpose(0, 1, 3, 2).copy()
    dl_t = dl_pad.reshape(ncores, nb, C, P).transpose(0, 1, 3, 2).copy()
    ea_t = (
        ea_pad.reshape(ncores, nb, C, P, DE)
        .transpose(0, 1, 4, 2, 3)
        .reshape(ncores, nb, DE, cap)
        .copy()
    )

    xp = np.zeros((npad, P), np.float32)
    xp[:N] = x
    xtf = np.ascontiguousarray(xp.T)  # [P, npad]

    # per-core batch-local graph ids + pooled-scatter targets
    bl_all = np.full(npad, 1.0e9, np.float32)
    bl_tiles = np.zeros((ncores, nb, P, 1), np.float32)
    pool_tgt = np.zeros((ncores, P, 1), np.int32)
    for k in range(ncores):
        g_base = int(batch[min(k * npc, N - 1)])
        sl = np.full(npc, 1.0e9, np.float32)
        hi = min((k + 1) * npc, N)
        if k * npc < N:
            sl[: hi - k * npc] = batch[k * npc : hi].astype(np.float32) - g_base
        span = int(sl[sl < P * 2].max()) + 1 if (sl < P * 2).any() else 0
        assert span <= P, f"graph span {span} > {P} on core {k}"
        bl_tiles[k] = sl.reshape(nb, P, 1)
        tgt = g_base + np.arange(P)
        oob = tgt >= G
        tgt[oob] = G + (np.arange(P)[oob] % (GP - G))
        pool_tgt[k, :, 0] = tgt.astype(np.int32)

    counts = np.bincount(batch, minlength=G).astype(np.float32)
    inv_counts = (1.0 / np.maximum(counts, 1.0)).astype(np.float32)
    gtiles = max(1, G // P) if G >= P else 1
    inv_c = np.zeros((gtiles, P, 1), np.float32)
    inv_c.reshape(-1)[:G] = inv_counts

    up = np.zeros((G, DG), np.float32)
    up[: u.shape[0]] = u
    u_T = np.ascontiguousarray(up.T)  # [DG, G]

    rep = {
        "x_T_full": xtf,
        "W_src": np.ascontiguousarray(We1[:P]),
        "W_dst": np.ascontiguousarray(We1[P : 2 * P]),
        "W_e": np.ascontiguousarray(We1[2 * P :]),
        "be1_row": be1.reshape(1, P),
        "We2": We2,
        "be2_row": be2.reshape(1, P),
        "Wg_x": np.ascontiguousarray(Wg[:P]),
        "Wg_a": np.ascontiguousarray(Wg[P:]),
        "bg_row": bg.reshape(1, P),
        "Wn_x": np.ascontiguousarray(Wn[:P]),
        "Wn_a": np.ascontiguousarray(Wn[P:]),
        "bn_row": bn.reshape(1, P),
        "Wu_u": np.ascontiguousarray(Wu[:DG]),
        "Wu_p": np.ascontiguousarray(Wu[DG:]),
        "bu_row": bu.reshape(1, DG),
        "u_T": u_T,
        "inv_counts": inv_c,
        "identity": np.eye(P, dtype=np.float32),
        "iota_row": np.broadcast_to(
            np.arange(P, dtype=np.float32), (P, P)
        ).copy(),
        "ones_row": np.ones((1, P), np.float32),
    }

    in_maps = []
    for k in range(ncores):
        m = dict(rep)
        m["x_T_local"] = np.ascontiguousarray(xtf[:, k * npc : (k + 1) * npc])
        m["src_idx"] = src_t[k]
        m["dst_idx"] = dg_t[k]
        m["dest_local"] = dl_t[k]
        m["ea_tiles"] = ea_t[k]
        m["batch_local"] = bl_tiles[k]
        m["pool_tgt"] = pool_tgt[k]
        in_maps.append(m)

    return in_maps, C, d


# --------------------------------------------------------------------------
# device program
# --------------------------------------------------------------------------

def build_program(C, dims):
    d = _derive(dims)
    ncores, npc, nb, npad = d["n_cores"], d["npc"], d["nb"], d["npad"]
    G, GP, DE, DG = d["n_graphs"], d["gp"], d["d_edge"], d["d_glob"]
    cap = C * P
    ntp = npad // P  # node tiles for the full table
    gtiles = max(1, G // P)
    gdim = min(G, P)

    nc = bacc.Bacc(
        "TRN2",
        target_bir_lowering=False,
        debug=False,
        enable_asserts=False,
        num_devices=ncores,
    )

    def din(name, shape, dt=F32):
        return nc.dram_tensor(name, list(shape), dt, kind="ExternalInput").ap()

    x_T_full = din("x_T_full", (P, npad))
    x_T_local = din("x_T_local", (P, npc))
    W_src = din("W_src", (P, P))
    W_dst = din("W_dst", (P, P))
    W_e = din("W_e", (DE, P))
    be1_row = din("be1_row", (1, P))
    We2 = din("We2", (P, P))
    be2_row = din("be2_row", (1, P))
    Wg_x = din("Wg_x", (P, P))
    Wg_a = din("Wg_a", (P, P))
    bg_row = din("bg_row", (1, P))
    Wn_x = din("Wn_x", (P, P))
    Wn_a = din("Wn_a", (P, P))
    bn_row = din("bn_row", (1, P))
    Wu_u = din("Wu_u", (DG, DG))
    Wu_p = din("Wu_p", (P, DG))
    bu_row = din("bu_row", (1, DG))
    u_T = din("u_T", (DG, G))
    inv_counts = din("inv_counts", (gtiles, P, 1))
    identity = din("identity", (P, P))
    iota_row = din("iota_row", (P, P))
    ones_row = din("ones_row", (1, P))
    src_idx = din("src_idx", (nb, P, C), I32)
    dst_idx = din("dst_idx", (nb, P, C), I32)
    dest_local = din("dest_local", (nb, P, C))
    ea_tiles = din("ea_tiles", (nb, DE, cap))
    batch_local = din("batch_local", (nb, P, 1))
    pool_tgt = din("pool_tgt", (P, 1), I32)

    xw_out = nc.dram_tensor("xw_out", [npc, P], F32, kind="ExternalOutput").ap()
    u_out = nc.dram_tensor("u_out", [G, DG], F32, kind="ExternalOutput").ap()

    h_src_tab = nc.dram_tensor("h_src_tab", [npad, P], F32).ap()
    h_dst_tab = nc.dram_tensor("h_dst_tab", [npad, P], F32).ap()

    mm = nc.tensor.matmul

    with tile.TileContext(nc) as tc, ExitStack() as ctx:
        cp = ctx.enter_context(tc.tile_pool(name="consts", bufs=1))

        def const(ap_in, shape, dt=F32, name="c"):
            t = cp.tile(shape, dt, name=name)
            nc.sync.dma_start(out=t[:], in_=ap_in[:])
            return t

        W_e_sb = const(W_e, (DE, P), name="W_e_sb")
        We2_sb = const(We2, (P, P), name="We2_sb")
        Wg_x_sb = const(Wg_x, (P, P), name="Wg_x_sb")
        Wg_a_sb = const(Wg_a, (P, P), name="Wg_a_sb")
        Wn_x_sb = const(Wn_x, (P, P), name="Wn_x_sb")
        Wn_a_sb = const(Wn_a, (P, P), name="Wn_a_sb")
        Wu_u_sb = const(Wu_u, (DG, DG), name="Wu_u_sb")
        Wu_p_sb = const(Wu_p, (P, DG), name="Wu_p_sb")
        W_src_sb = const(W_src, (P, P), name="W_src_sb")
        W_dst_sb = const(W_dst, (P, P), name="W_dst_sb")
        be1_sb = const(be1_row, (1, P), name="be1_sb")
        be2_sb = const(be2_row, (1, P), name="be2_sb")
        bg_sb = const(bg_row, (1, P), name="bg_sb")
        bn_sb = const(bn_row, (1, P), name="bn_sb")
        bu_sb = const(bu_row, (1, DG), name="bu_sb")
        ident_sb = const(identity, (P, P), name="ident_sb")
        iota_sb = const(iota_row, (P, P), name="iota_sb")
        ones_sb = const(ones_row, (1, P), name="ones_sb")
        u_T_sb = const(u_T, (DG, G), name="u_T_sb")

        # ---------------- phase 0: h tables --------------------------------
        with ExitStack() as c0:
            p0s = c0.enter_context(tc.tile_pool(name="p0s", bufs=4))
            p0o = c0.enter_context(tc.tile_pool(name="p0o", bufs=4))
            p0p = c0.enter_context(tc.tile_pool(name="p0p", bufs=4, space="PSUM"))
            for i in range(ntp):
                xt = p0s.tile([P, P], F32, name="xt")
                nc.sync.dma_start(out=xt[:], in_=x_T_full[:, i * P : (i + 1) * P])
                hp = p0p.tile([P, 2 * P], F32, name="hp")
                mm(out=hp[:, :P], lhsT=xt[:], rhs=W_src_sb[:], start=True, stop=True)
                mm(out=hp[:, P:], lhsT=xt[:], rhs=W_dst_sb[:], start=True, stop=False)
                mm(out=hp[:, P:], lhsT=ones_sb[:], rhs=be1_sb[:], start=False, stop=True)
                hs = p0o.tile([P, P], F32, name="hs")
                hd = p0o.tile([P, P], F32, name="hd")
                nc.scalar.activation(out=hs[:], in_=hp[:, :P], func=AF.Copy)
                nc.vector.tensor_copy(out=hd[:], in_=hp[:, P:])
                nc.sync.dma_start(out=h_src_tab[i * P : (i + 1) * P, :], in_=hs[:])
                nc.sync.dma_start(out=h_dst_tab[i * P : (i + 1) * P, :], in_=hd[:])

        # ---------------- phase 1+2: edges + nodes -------------------------
        with ExitStack() as c1:
            sA = c1.enter_context(tc.tile_pool(name="sA", bufs=4))
            sB = c1.enter_context(tc.tile_pool(name="sB", bufs=2))
            zp = c1.enter_context(tc.tile_pool(name="zp", bufs=2, space="PSUM"))
            mp = c1.enter_context(tc.tile_pool(name="mp", bufs=2, space="PSUM"))
            ap_ = c1.enter_context(tc.tile_pool(name="ap", bufs=1, space="PSUM"))
            tp = c1.enter_context(tc.tile_pool(name="tp", bufs=2, space="PSUM"))
            pp = c1.enter_context(tc.tile_pool(name="pp", bufs=1, space="PSUM"))
            dp = c1.enter_context(tc.tile_pool(name="dp", bufs=1, space="DRAM"))

            pool_ps = pp.tile([P, P], F32, name="pool_ps")

            for b in range(nb):
                idxs = sB.tile([P, C], I32, name="idxs")
                idxd = sB.tile([P, C], I32, name="idxd")
                dls = sB.tile([P, C], F32, name="dls")
                eas = sB.tile([DE, cap], F32, name="eas")
                nc.sync.dma_start(out=idxs[:], in_=src_idx[b])
                nc.sync.dma_start(out=idxd[:], in_=dst_idx[b])
                nc.sync.dma_start(out=dls[:], in_=dest_local[b])
                nc.sync.dma_start(out=eas[:], in_=ea_tiles[b])
                agg_ps = ap_.tile([P, P], F32, name="agg_ps")
                for t in range(C):
                    hsum = sA.tile([P, P], F32, name="hsum")
                    nc.gpsimd.indirect_dma_start(
                        out=hsum[:],
                        out_offset=None,
                        in_=h_src_tab[:],
                        in_offset=IndirectOffsetOnAxis(ap=idxs[:, t : t + 1], axis=0),
                    )
                    nc.gpsimd.indirect_dma_start(
                        out=hsum[:],
                        out_offset=None,
                        in_=h_dst_tab[:],
                        in_offset=IndirectOffsetOnAxis(ap=idxd[:, t : t + 1], axis=0),
                        compute_op=ALU.add,
                    )
                    z_ps = zp.tile([P, P], F32, name="z_ps", tag="zp")
                    mm(out=z_ps[:], lhsT=W_e_sb[:], rhs=eas[:, t * P : (t + 1) * P],
                       start=True, stop=False)
                    mm(out=z_ps[:], lhsT=hsum[:], rhs=ident_sb[:],
                       start=False, stop=True)
                    a1T = sA.tile([P, P], F32, name="a1T")
                    nc.scalar.activation(out=a1T[:], in_=z_ps[:], func=AF.Relu)
                    m_ps = mp.tile([P, P], F32, name="m_ps", tag="mp")
                    mm(out=m_ps[:], lhsT=a1T[:], rhs=We2_sb[:], start=True, stop=False)
                    mm(out=m_ps[:], lhsT=ones_sb[:], rhs=be2_sb[:], start=False, stop=True)
                    msg = sA.tile([P, P], F32, name="msg")
                    nc.vector.tensor_scalar_max(out=msg[:], in0=m_ps[:], scalar1=0.0)
                    S = sA.tile([P, P], F32, name="S")
                    nc.vector.tensor_tensor(
                        out=S[:],
                        in0=dls[:, t : t + 1].to_broadcast([P, P]),
                        in1=iota_sb[:],
                        op=ALU.is_equal,
                    )
                    mm(out=agg_ps[:], lhsT=S[:], rhs=msg[:],
                       start=(t == 0), stop=(t == C - 1))

                # node update for block b
                agg_sb = sB.tile([P, P], F32, name="agg_sb")
                nc.scalar.activation(out=agg_sb[:], in_=agg_ps[:], func=AF.Copy)
                t_ps = tp.tile([P, P], F32, name="t_ps", tag="tp")
                nc.tensor.transpose(out=t_ps[:], in_=agg_sb[:], identity=ident_sb[:])
                aggT = sB.tile([P, P], F32, name="aggT")
                nc.vector.tensor_copy(out=aggT[:], in_=t_ps[:])
                xt2 = sB.tile([P, P], F32, name="xt2")
                nc.sync.dma_start(out=xt2[:], in_=x_T_local[:, b * P : (b + 1) * P])
                g_ps = zp.tile([P, P], F32, name="g_ps", tag="zp")
                mm(out=g_ps[:], lhsT=xt2[:], rhs=Wg_x_sb[:], start=True, stop=False)
                mm(out=g_ps[:], lhsT=aggT[:], rhs=Wg_a_sb[:], start=False, stop=False)
                mm(out=g_ps[:], lhsT=ones_sb[:], rhs=bg_sb[:], start=False, stop=True)
                gate = sB.tile([P, P], F32, name="gate")
                nc.scalar.activation(out=gate[:], in_=g_ps[:], func=AF.Sigmoid)
                nc.sync.dma_start(out=xw_out[b * P : (b + 1) * P, :], in_=gate[:])
                n_ps = mp.tile([P, P], F32, name="n_ps", tag="mp")
                mm(out=n_ps[:], lhsT=xt2[:], rhs=Wn_x_sb[:], start=True, stop=False)
                mm(out=n_ps[:], lhsT=aggT[:], rhs=Wn_a_sb[:], start=False, stop=False)
                mm(out=n_ps[:], lhsT=ones_sb[:], rhs=bn_sb[:], start=False, stop=True)
                rn = sB.tile([P, P], F32, name="rn")
                nc.vector.tensor_scalar_max(out=rn[:], in0=n_ps[:], scalar1=0.0)
                xn = sB.tile([P, P], F32, name="xn")
                nc.vector.tensor_mul(out=xn[:], in0=gate[:], in1=rn[:])
                blt = sB.tile([P, 1], F32, name="blt")
                nc.sync.dma_start(out=blt[:], in_=batch_local[b])
                Sg = sB.tile([P, P], F32, name="Sg")
                nc.vector.tensor_tensor(
                    out=Sg[:],
                    in0=blt[:, :1].to_broadcast([P, P]),
                    in1=iota_sb[:],
                    op=ALU.is_equal,
                )
                mm(out=pool_ps[:], lhsT=Sg[:], rhs=xn[:],
                   start=(b == 0), stop=(b == nb - 1))

            # ---------------- pooled partials + all-reduce -----------------
            pool_sb = sB.tile([P, P], F32, name="pool_sb")
            nc.vector.tensor_copy(out=pool_sb[:], in_=pool_ps[:])
            zer = sB.tile([P, P], F32, name="zer")
            nc.gpsimd.memset(zer[:], 0.0)
            pp_in = dp.tile([GP, P], F32, name="pp_in")
            pp_out = dp.tile([GP, P], F32, name="pp_out")
            for r in range(0, GP, P):
                rows = min(P, GP - r)
                nc.sync.dma_start(out=pp_in[r : r + rows, :], in_=zer[:rows, :])
            ptgt = sB.tile([P, 1], I32, name="ptgt")
            nc.sync.dma_start(out=ptgt[:], in_=pool_tgt[:])
            nc.gpsimd.indirect_dma_start(
                out=pp_in[:],
                out_offset=IndirectOffsetOnAxis(ap=ptgt[:, :1], axis=0),
                in_=pool_sb[:],
                in_offset=None,
            )
            nc.gpsimd.collective_compute(
                "AllReduce",
                ALU.add,
                replica_groups=[list(range(ncores))],
                ins=[pp_in[:]],
                outs=[pp_out[:]],
            )

            # ---------------- phase 3: global MLP --------------------------
            for gt in range(gtiles):
                pr = sB.tile([P, P], F32, name="pr")
                if gdim < P:
                    nc.gpsimd.memset(pr[:], 0.0)
                nc.sync.dma_start(out=pr[:gdim, :], in_=pp_out[gt * P : gt * P + gdim, :])
                ic = sB.tile([P, 1], F32, name="ic")
                nc.sync.dma_start(out=ic[:], in_=inv_counts[gt])
                psc = sB.tile([P, P], F32, name="psc")
                nc.vector.tensor_scalar_mul(out=psc[:], in0=pr[:], scalar1=ic[:, :1])
                t3 = tp.tile([P, P], F32, name="t3", tag="tp")
                nc.tensor.transpose(out=t3[:], in_=psc[:], identity=ident_sb[:])
                plT = sB.tile([P, P], F32, name="plT")
                nc.vector.tensor_copy(out=plT[:], in_=t3[:])
                u_ps = zp.tile([P, DG], F32, name="u_ps", tag="zp")
                mm(out=u_ps[:gdim, :], lhsT=u_T_sb[:, gt * P : gt * P + gdim],
                   rhs=Wu_u_sb[:], start=True, stop=False)
                mm(out=u_ps[:gdim, :], lhsT=plT[:, :gdim], rhs=Wu_p_sb[:],
                   start=False, stop=False)
                mm(out=u_ps[:gdim, :], lhsT=ones_sb[:, :gdim], rhs=bu_sb[:],
                   start=False, stop=True)
                un = sB.tile([P, DG], F32, name="un")
                nc.scalar.activation(out=un[:gdim, :], in_=u_ps[:gdim, :], func=AF.Relu)
                nc.sync.dma_start(out=u_out[gt * P : gt * P + gdim, :], in_=un[:gdim, :])

    nc.compile()
    return nc


_PROG_CACHE = {}


def _get_program(C, dims):
    key = (C, tuple(sorted(dims.items())))
    if key not in _PROG_CACHE:
        _PROG_CACHE[key] = build_program(C, dims)
    return _PROG_CACHE[key]


def assemble_outputs(results, dims):
    d = _derive(dims)
    N, npc = d["n_nodes"], d["npc"]
    xw = np.concatenate([r["xw_out"] for r in results], axis=0)[:N]
    u_new = results[0]["u_out"][: d["n_graphs"]]
    return u_new, xw


def run(inputs, dims=None, trace=False):
    dims = dims or default_dims()
    in_maps, C, d = prep_inputs(inputs, dims)
    nc = _get_program(C, dims)
    res = run_bass_kernel_spmd(
        nc, in_maps, core_ids=list(range(d["n_cores"])), trace=trace
    )
    u_new, xw = assemble_outputs(res.results, dims)
    return (u_new, xw), res


def kernel(**inputs):
    (u_new, xw), _ = run(inputs)
    return u_new, xw
